# revision 1
# baseline (speedup 1.0000x reference)
"""Trainium2 Bass kernel for multi-head GQA attention (dense transformer layer).

Problem: x[2,2048,4096] -> attention(RoPE, GQA 32q/8kv heads, causal) -> out[2,2048,4096]

Strategy (8 NeuronCores, tensor-parallel by heads):
  - Core c owns q-heads 4c..4c+3 and kv-head c (wq/wk/wv column shards).
  - Everything on device is computed in "feature-on-partition" layout:
      activations X^T [din, tok], Q^T/K^T [d, tok], scores^T [k, q].
    This makes softmax denominators a ones-matmul / DVE-add and avoids all
    transposes of the probability tiles.
  - Softmax skips the running-max (scores are O(10) here; exp is safe in fp32).
  - Attention outputs (4 heads per core, [512, 2048] bf16 per batch) are
    AllGathered on the partition axis (one AG per batch, overlapping compute)
    -> every core holds attn^T [4096, 2048]; each core then computes a
    512-column slice of the output projection (wo column shard), so no
    AllReduce is needed; host concatenates + transposes.
  - Matmuls in bf16 with fp32 PSUM accumulation; RoPE tables/masks in bf16.
"""

import numpy as np
from contextlib import ExitStack

import concourse.bass as bass
import concourse.tile as tile
from concourse import bacc, mybir
from concourse.bass import ts
from concourse.bass_utils import run_bass_kernel_spmd

BF16 = mybir.dt.bfloat16
F32 = mybir.dt.float32

N_CORES = 8
DIM = 4096
N_HEADS = 32
HEAD_DIM = 128
BATCH = 2
SEQ = 2048

TOK = BATCH * SEQ            # 4096 tokens, batch-major
NB = TOK // 512              # 8 token blocks of 512
NBB = SEQ // 512             # 4 token blocks per batch
KT = DIM // 128              # 32 contraction tiles for the projections
H_PER_CORE = N_HEADS // N_CORES       # 4
DQ = H_PER_CORE * HEAD_DIM            # 512 q-dims per core
QB = SEQ // 512              # 4 query blocks of 512 per batch
SKT = SEQ // 128             # 16 key tiles of 128 per batch

EXP = mybir.ActivationFunctionType.Exp


def build_program(reps: int = 1, n_cores: int = N_CORES,
                  collective: bool = True) -> bass.Bass:
    nc = bacc.Bacc("TRN2", target_bir_lowering=False, debug=False,
                   num_devices=n_cores)

    # ---- I/O (per-core tensors; host pre-arranges layouts) ----
    xT = nc.dram_tensor("xT", [KT, 128, TOK], BF16, kind="ExternalInput").ap()
    wq = nc.dram_tensor("wq", [128, KT * DQ], BF16, kind="ExternalInput").ap()
    wk = nc.dram_tensor("wk", [128, KT * 128], BF16, kind="ExternalInput").ap()
    wv = nc.dram_tensor("wv", [128, KT * 128], BF16, kind="ExternalInput").ap()
    wo = nc.dram_tensor("wo", [128, KT * DQ], BF16, kind="ExternalInput").ap()
    cosT = nc.dram_tensor("cosT", [128, SEQ], BF16, kind="ExternalInput").ap()
    sinT = nc.dram_tensor("sinT", [128, SEQ], BF16, kind="ExternalInput").ap()
    pmat = nc.dram_tensor("pmat", [128, 128], BF16, kind="ExternalInput").ap()
    tri = nc.dram_tensor("tri", [128, 128], BF16, kind="ExternalInput").ap()
    ident = nc.dram_tensor("ident", [128, 128], BF16, kind="ExternalInput").ap()
    ones = nc.dram_tensor("ones", [128, 1], BF16, kind="ExternalInput").ap()
    outT = nc.dram_tensor("outT", [DQ, TOK], F32, kind="ExternalOutput").ap()

    # internal DRAM for the collectives (cannot use I/O tensors); one pair
    # per (rep, batch) so reps never race and batches can overlap.
    cc_in = [[nc.dram_tensor(f"cc_in{r}_{b}", [DQ, SEQ], BF16)
              for b in range(BATCH)] for r in range(reps)]
    cc_out = [[nc.dram_tensor(f"cc_out{r}_{b}", [N_HEADS * HEAD_DIM, SEQ], BF16,
                              addr_space="Shared")
               for b in range(BATCH)] for r in range(reps)]

    with tile.TileContext(nc) as tc, ExitStack() as top:
        consts = top.enter_context(tc.tile_pool(name="consts", bufs=1))
        weights = top.enter_context(tc.tile_pool(name="weights", bufs=1))
        acts = top.enter_context(tc.tile_pool(name="acts", bufs=1))

        # weights: the very first matmuls gate only on the k=0..7 chunks;
        # everything else is emitted right after the first x-tile DMA (see
        # the late_loads callback below) to keep the HWDGE trigger queue
        # prefix minimal.
        wq_sb = weights.tile([128, KT, DQ], BF16)
        wk_sb = weights.tile([128, KT, 128], BF16)
        wv_sb = weights.tile([128, KT, 128], BF16)
        wo_sb = weights.tile([128, KT, DQ], BF16)
        wq3 = wq.rearrange("p (k m) -> p k m", k=KT)
        wk3 = wk.rearrange("p (k m) -> p k m", k=KT)
        wv3 = wv.rearrange("p (k m) -> p k m", k=KT)
        ck = KT // 4
        ksl = slice(0, ck)
        nc.sync.dma_start(wq_sb[:, ksl, :], wq3[:, ksl, :])
        nc.sync.dma_start(wk_sb[:, ksl, :], wk3[:, ksl, :])
        nc.sync.dma_start(wv_sb[:, ksl, :], wv3[:, ksl, :])

        # tiny constants next
        pm_sb = consts.tile([128, 128], BF16)
        nc.sync.dma_start(pm_sb[:], pmat[:, :])
        tri_sb = consts.tile([128, 128], BF16)
        nc.sync.dma_start(tri_sb[:], tri[:, :])
        id_sb = consts.tile([128, 128], BF16)
        nc.sync.dma_start(id_sb[:], ident[:, :])
        ones_sb = consts.tile([128, 1], BF16)
        nc.sync.dma_start(ones_sb[:], ones[:, :])

        cos_sb = consts.tile([128, SEQ], BF16)
        sin_sb = consts.tile([128, SEQ], BF16)

        def late_loads():
            ksl = slice(ck, KT)
            nc.sync.dma_start(wq_sb[:, ksl, :], wq3[:, ksl, :])
            nc.sync.dma_start(wk_sb[:, ksl, :], wk3[:, ksl, :])
            nc.sync.dma_start(wv_sb[:, ksl, :], wv3[:, ksl, :])
            nc.sync.dma_start(cos_sb[:], cosT[:, :])
            nc.sync.dma_start(sin_sb[:], sinT[:, :])
        run_body.late_loads = late_loads

        # per-core activations (feature-major / layout B)
        qt_sb = [acts.tile([128, TOK], BF16, tag=f"qt{m}", name=f"qt{m}")
                 for m in range(H_PER_CORE)]
        kt_sb = acts.tile([128, TOK], BF16)
        vt_sb = acts.tile([128, TOK], BF16)          # V^T, pre-transpose
        va_sb = acts.tile([128, KT, 128], BF16)      # V in [tok, dv] tiles

        run_body.wo_dram = wo
        for rep in range(reps):
            run_body(nc, tc, rep, cc_in[rep], cc_out[rep], outT,
                     wq_sb, wk_sb, wv_sb, wo_sb, cos_sb, sin_sb, pm_sb,
                     tri_sb, id_sb, ones_sb, qt_sb, kt_sb, vt_sb, va_sb, xT,
                     n_cores=n_cores, collective=collective)

    nc.compile()
    return nc


def run_body(nc, tc, rep, cc_in, cc_out, outT,
             wq_sb, wk_sb, wv_sb, wo_sb, cos_sb, sin_sb, pm_sb,
             tri_sb, id_sb, ones_sb, qt_sb, kt_sb, vt_sb, va_sb, xT,
             n_cores=N_CORES, collective=True):
    # ---------------- phase 1: QKV projection + RoPE ----------------
    with ExitStack() as body:
      ps = body.enter_context(tc.tile_pool(name=f"ps_{rep}", bufs=1, space="PSUM"))
      with ExitStack() as ph:
        xin = ph.enter_context(tc.tile_pool(name=f"xin{rep}", bufs=8))
        rope = ph.enter_context(tc.tile_pool(name=f"rope{rep}", bufs=3))

        for n in range(NB):
            s0 = n % QB                 # 512-block position within the batch
            q_ps = [ps.tile([128, 512], F32, tag=f"qps{m}", bufs=1, name=f"qps{m}")
                    for m in range(H_PER_CORE)]
            k_ps = ps.tile([128, 512], F32, tag="kps", bufs=1)
            v_ps = ps.tile([128, 512], F32, tag="vps", bufs=1)
            for k in range(KT):
                xt = xin.tile([128, 512], BF16, tag="xt")
                nc.sync.dma_start(xt[:], xT[k, :, ts(n, 512)])
                if rep == 0 and n == 0 and k == 8 and run_body.late_loads:
                    run_body.late_loads()
                    run_body.late_loads = None
                st, sp = (k == 0), (k == KT - 1)
                for m in range(H_PER_CORE):
                    nc.tensor.matmul(q_ps[m][:], wq_sb[:, k, ts(m, 128)],
                                     xt[:], start=st, stop=sp)
                nc.tensor.matmul(k_ps[:], wk_sb[:, k, :], xt[:], start=st, stop=sp)
                nc.tensor.matmul(v_ps[:], wv_sb[:, k, :], xt[:], start=st, stop=sp)

            # V^T: plain copy out of PSUM
            nc.scalar.copy(vt_sb[:, ts(n, 512)], v_ps[:])

            # RoPE on Q heads and K:  y = raw*cos + (P@raw)*sin (scale folded in)
            def do_rope(acc, dst):
                raw = rope.tile([128, 512], BF16, tag="raw")
                nc.scalar.copy(raw[:], acc[:])
                rot = ps.tile([128, 512], F32, tag="rot", bufs=2)
                nc.tensor.matmul(rot[:], pm_sb[:], raw[:], start=True, stop=True)
                t1 = rope.tile([128, 512], BF16, tag="t1")
                nc.vector.tensor_mul(t1[:], raw[:], cos_sb[:, ts(s0, 512)])
                t2 = rope.tile([128, 512], BF16, tag="t2")
                nc.vector.tensor_mul(t2[:], rot[:], sin_sb[:, ts(s0, 512)])
                nc.vector.tensor_add(dst, t1[:], t2[:])

            for m in range(H_PER_CORE):
                do_rope(q_ps[m], qt_sb[m][:, ts(n, 512)])
            do_rope(k_ps, kt_sb[:, ts(n, 512)])

            # V^T -> V transposes for this block (shares the rot psum slots)
            for t in range(4 * n, 4 * n + 4):
                tr = ps.tile([128, 128], BF16, tag="rot", bufs=2, name="tr")
                nc.tensor.transpose(tr[:], vt_sb[:, ts(t, 128)], id_sb[:])
                nc.vector.tensor_copy(va_sb[:, t, :], tr[:])

      # ------- phase 2+3: attention + AG (overlapped), then wo -------
      if rep == 0:
        wo3 = run_body.wo_dram.rearrange("p (k m) -> p k m", k=KT)
        for c4 in range(4):
            ksl = slice(c4 * (KT // 4), (c4 + 1) * (KT // 4))
            nc.sync.dma_start(wo_sb[:, ksl, :], wo3[:, ksl, :])
      with ExitStack() as ph:
        work = ph.enter_context(tc.tile_pool(name=f"attnwork{rep}", bufs=4))

        def attention_batch(b):
            for qb in range(QB):
                gq = b * SEQ + qb * 512
                nkt = (qb + 1) * 4
                atags = ["qps3", "kps", "vps", "rot"]
                o_ps = [ps.tile([128, 512], F32, tag=atags[h],
                                bufs=(2 if atags[h] == "rot" else 1),
                                name=f"aops{h}") for h in range(H_PER_CORE)]
                # one psum bank holds all 4 heads' softmax denominators at
                # 32-aligned partition offsets (col-group packed matmuls)
                dn_ps = ps.tile([128, 512], F32, tag="rot", bufs=2, name="dn")
                ex_prev = [None] * H_PER_CORE
                off_prev = [0] * H_PER_CORE
                for kt in range(nkt):
                    gk = b * SEQ + kt * 128
                    vtile = b * SKT + kt
                    j = kt - qb * 4          # >= 0 -> diagonal tile
                    q_off = 128 * j if j > 0 else 0
                    N = 512 - q_off
                    st, sp = (kt == 0), (kt == nkt - 1)
                    for h in range(H_PER_CORE):
                        s_ps = ps.tile([128, 512], F32,
                                       tag=f"qps{(kt * 4 + h) % 3}", bufs=1,
                                       name="sps")
                        nc.tensor.matmul(s_ps[:, :N], kt_sb[:, gk:gk + 128],
                                         qt_sb[h][:, gq + q_off:gq + 512],
                                         start=True, stop=True)
                        ex = work.tile([128, 512], BF16, tag="expT", bufs=12,
                                       name="ex")
                        nc.scalar.activation(ex[:, :N], s_ps[:, :N], EXP)
                        if j >= 0:
                            nc.vector.tensor_mul(ex[:, :128], ex[:, :128],
                                                 tri_sb[:])
                        nc.tensor.matmul(o_ps[h][:, q_off:], va_sb[:, vtile, :],
                                         ex[:, :N], start=st, stop=sp)
                        if kt % 2 == 0:
                            ex_prev[h] = ex
                            off_prev[h] = q_off
                        else:
                            # fold this tile's exp sums into the previous
                            # tile (bf16 add), one denominator matmul per pair
                            exp_, offp = ex_prev[h], off_prev[h]
                            d = q_off - offp
                            nc.vector.tensor_add(exp_[:, d:512 - offp],
                                                 exp_[:, d:512 - offp],
                                                 ex[:, :N])
                            nc.tensor.matmul(
                                dn_ps[32 * h:32 * h + 1, offp:],
                                ones_sb[:], exp_[:, :512 - offp],
                                start=(kt == 1), stop=(kt == nkt - 1),
                                tile_position=(0, 32 * h))
                            ex_prev[h] = None
                for h in range(H_PER_CORE):
                    oc = work.tile([128, 512], F32, tag="oc", name="oc")
                    nc.vector.tensor_copy(oc[:], o_ps[h][:])   # frees psum bank fast
                    rec = work.tile([1, 512], F32, tag="rec", name="rec")
                    nc.vector.reciprocal(rec[:], dn_ps[32 * h:32 * h + 1, :])
                    rbc = work.tile([128, 512], F32, tag="rbc", name="rbc")
                    nc.gpsimd.partition_broadcast(rbc[:], rec[:])
                    at = work.tile([128, 512], BF16, tag="at", name="at")
                    nc.vector.tensor_mul(at[:], oc[:], rbc[:])
                    nc.sync.dma_start(
                        cc_in[b].ap()[h * 128:(h + 1) * 128, ts(qb, 512)], at[:])

        def gather_batch(b):
            if collective:
                nc.gpsimd.collective_compute(
                    "AllGather",
                    mybir.AluOpType.bypass,
                    ins=[cc_in[b].ap().opt()],
                    outs=[cc_out[b].ap().opt()],
                    replica_groups=[list(range(n_cores))],
                )

        ain = ph.enter_context(tc.tile_pool(name=f"ain{rep}", bufs=12))

        def wo_batch(b):
            for nl in range(NBB):
                n = b * NBB + nl
                wtags = ["qps3", "kps", "vps", "rot"]
                o_ps = [ps.tile([128, 512], F32, tag=wtags[m],
                                bufs=(2 if wtags[m] == "rot" else 1),
                                name=f"wops{m}") for m in range(H_PER_CORE)]
                for k in range(KT):
                    at = ain.tile([128, 512], BF16, tag="woat", name="woat")
                    eng = nc.scalar if k % 2 == 0 else nc.sync
                    eng.dma_start(at[:], cc_out[b].ap()[ts(k, 128), ts(nl, 512)])
                    st, sp = (k == 0), (k == KT - 1)
                    for m in range(H_PER_CORE):
                        nc.tensor.matmul(o_ps[m][:], wo_sb[:, k, ts(m, 128)],
                                         at[:], start=st, stop=sp)
                for m in range(H_PER_CORE):
                    ot = ain.tile([128, 512], F32, tag="ot", name="ot", bufs=4)
                    if m % 2 == 0:
                        nc.scalar.copy(ot[:], o_ps[m][:])
                    else:
                        nc.vector.tensor_copy(ot[:], o_ps[m][:])
                    nc.scalar.dma_start(outT[ts(m, 128), ts(n, 512)], ot[:])

        attention_batch(0)
        gather_batch(0)
        attention_batch(1)
        gather_batch(1)
        wo_batch(0)
        wo_batch(1)


def prepare_inputs(x, cos, sin, wq, wk, wv, wo):
    """Host-side: slice/transpose/cast all per-core arrays."""
    import ml_dtypes
    s4 = float(HEAD_DIM) ** -0.25

    xT = np.ascontiguousarray(
        x.reshape(TOK, DIM).T.reshape(KT, 128, TOK)).astype(np.float32)

    cosT = np.ascontiguousarray(cos.T) * s4     # [128, SEQ]
    sinT = np.ascontiguousarray(sin.T) * s4

    # rotate-half matrix: (P @ u) = [-u2; u1];  lhsT = P^T
    P = np.zeros((128, 128), np.float32)
    for d in range(64):
        P[d, d + 64] = -1.0
        P[d + 64, d] = 1.0
    PT = P.T.copy()

    ones = np.ones((128, 1), np.float32)

    # diagonal-block mask for scores^T [k, q]: valid iff k <= q
    kk = np.arange(128)[:, None]
    qq = np.arange(128)[None, :]
    tri = (kk <= qq).astype(np.float32)

    def wslices(w, rows_per_core):
        # w: [out, DIM] -> per-core [128, KT, rows_per_core] (lhsT tiles)
        out = []
        for c in range(N_CORES):
            wc = w[c * rows_per_core:(c + 1) * rows_per_core, :]      # [R, DIM]
            wt = wc.T.reshape(KT, 128, rows_per_core).transpose(1, 0, 2)
            out.append(np.ascontiguousarray(wt).reshape(128, KT * rows_per_core))
        return out

    wq_c = wslices(wq, DQ)
    wk_c = wslices(wk, 128)
    wv_c = wslices(wv, 128)

    # wo: [DIM, N_HEADS*HEAD_DIM]; core c computes output columns 512c..512c+512
    wo_c = []
    for c in range(N_CORES):
        woc = wo[c * DQ:(c + 1) * DQ, :]            # [DQ out-rows, DIM hd]
        wt = woc.T.reshape(KT, 128, DQ).transpose(1, 0, 2)
        wo_c.append(np.ascontiguousarray(wt).reshape(128, KT * DQ))

    bf = lambda a: np.asarray(a, np.float32).astype(ml_dtypes.bfloat16)

    in_maps = []
    for c in range(N_CORES):
        in_maps.append({
            "xT": bf(xT.reshape(KT, 128, TOK)),
            "wq": bf(wq_c[c]),
            "wk": bf(wk_c[c]),
            "wv": bf(wv_c[c]),
            "wo": bf(wo_c[c]),
            "cosT": bf(cosT),
            "sinT": bf(sinT),
            "pmat": bf(PT),
            "tri": bf(tri),
            "ident": bf(np.eye(128, dtype=np.float32)),
            "ones": bf(ones),
        })
    return in_maps


_cached = {}


def _get_program():
    if "nc" not in _cached:
        _cached["nc"] = build_program()
    return _cached["nc"]


def kernel(x, cos, sin, wq, wk, wv, wo, start_pos):
    assert int(start_pos) == 0
    nc = _get_program()
    in_maps = prepare_inputs(np.asarray(x, np.float32), np.asarray(cos, np.float32),
                             np.asarray(sin, np.float32), np.asarray(wq, np.float32),
                             np.asarray(wk, np.float32), np.asarray(wv, np.float32),
                             np.asarray(wo, np.float32))
    res = run_bass_kernel_spmd(nc, in_maps, core_ids=list(range(N_CORES)))
    # outT per core: [512 do, 4096 tok]; concat -> [4096 do, 4096 tok]
    full = np.concatenate([res.results[c]["outT"] for c in range(N_CORES)], axis=0)
    out = full.T.reshape(BATCH, SEQ, DIM)
    return np.ascontiguousarray(out, dtype=np.float32)

